# revision 1
# baseline (speedup 1.0000x reference)
"""AttnBlock3d (GroupNorm -> QKV -> softmax attention -> proj -> residual) on 8 trn2 cores.

Sharding: 8 shards = batch (2) x query-chunk (4 x 1024 tokens). Each core receives the
full batch slice (for GN stats and K/V) plus its query chunk; per-core difference is
entirely in the input data, so one SPMD NEFF runs on all 8 cores with no collectives.
Host gathers the per-core [C, 1024] outputs back into [2, C, 16, 16, 16].

The GN affine is folded into the QKV weights (k-bias drops out of softmax exactly;
v-bias is applied post-normalization via the proj input). Softmax denominators come
free from a ones-column appended to V^T in the P^T.T @ [V^T|1] matmul.
"""

import ml_dtypes
import numpy as np

import concourse.bacc as bacc
import concourse.mybir as mybir
import concourse.tile as tile
from concourse.bass_utils import run_bass_kernel_spmd

B = 2
C = 256
G = 32
N = 4096          # D*H*W tokens per batch
NQ = 1024         # query chunk per core
EPS = 1e-5
SCALE = 1.0 / 16.0  # C ** -0.5
F32 = mybir.dt.float32
BF16 = mybir.dt.bfloat16
FP8 = mybir.dt.float8e4
NT = N // 128     # 32 nk tiles
NQT = NQ // 128   # 8 query tiles per core
WARMUP_MMS = 30   # keep PE busy (and HAM warm) while input DMA streams in

# vecs layout along the free dim: gamma, beta, bq, bv, bp
VG, VB, VBQ, VBV, VBP = range(5)


def build_nc():
    nc = bacc.Bacc("TRN2", target_bir_lowering=False, debug=False, num_devices=8)

    xb = nc.dram_tensor("xb", [2, 4, 128, 1024], FP8, kind="ExternalInput").ap()
    xqb = nc.dram_tensor("xqb", [C, NQ], FP8, kind="ExternalInput").ap()
    xq = nc.dram_tensor("xq", [C, NQ], F32, kind="ExternalInput").ap()
    wqT = nc.dram_tensor("wqT", [C, C], F32, kind="ExternalInput").ap()
    wkT = nc.dram_tensor("wkT", [C, C], F32, kind="ExternalInput").ap()
    wvT = nc.dram_tensor("wvT", [C, C], F32, kind="ExternalInput").ap()
    wpT = nc.dram_tensor("wpT", [C, C], F32, kind="ExternalInput").ap()
    vecs = nc.dram_tensor("vecs", [2, 128, 5], F32, kind="ExternalInput").ap()
    ig = nc.dram_tensor("ig", [2, 128, G], F32, kind="ExternalInput").ap()
    igt = nc.dram_tensor("igt", [G, C], F32, kind="ExternalInput").ap()
    y = nc.dram_tensor("y", [2, 2, 128, 512], F32, kind="ExternalOutput").ap()

    from concourse.masks import make_identity

    with tile.TileContext(nc) as tc:
        with (
            tc.tile_pool(name="consts", bufs=1) as consts,
            tc.tile_pool(name="small", bufs=1) as small,
            tc.tile_pool(name="kqv", bufs=1) as kqv,
            tc.tile_pool(name="attn", bufs=1) as attn,
        ):
            # ---- x DMAs first: they gate everything ----
            x_bf = [kqv.tile([128, N], FP8, tag=f"xbf{t}", name=f"xbf{t}") for t in range(2)]
            for t in range(2):
                for ch in range(4):
                    sl = slice(ch * 1024, (ch + 1) * 1024)
                    nc.sync.dma_start(out=x_bf[t][:, sl], in_=xb[t, ch])
            xq_bf = [kqv.tile([128, NQ], FP8, tag=f"xqbf{t}", name=f"xqbf{t}") for t in range(2)]
            for t in range(2):
                nc.sync.dma_start(out=xq_bf[t], in_=xqb[t * 128:(t + 1) * 128, :])

            # small constants + weights ride the gpsimd queue in parallel
            vecs_t = [consts.tile([128, 5], F32, tag=f"vecs{t}", name=f"vecs{t}") for t in range(2)]
            ig_t = [consts.tile([128, G], F32, tag=f"ig{t}", name=f"ig{t}") for t in range(2)]
            igt_sb = consts.tile([G, C], F32, tag="igt", name="igt")
            ident = consts.tile([128, 128], BF16, tag="ident", name="ident")
            warm_rhs = consts.tile([128, 512], BF16, tag="warm", name="warm")
            make_identity(nc, ident)
            nc.gpsimd.memset(warm_rhs, 0.25)
            for t in range(2):
                nc.gpsimd.dma_start(out=vecs_t[t], in_=vecs[t])
                nc.gpsimd.dma_start(out=ig_t[t], in_=ig[t])
            nc.gpsimd.dma_start(out=igt_sb, in_=igt)

            wbf = {}   # folded bf16 weights, [c-tile][128, 256]
            for w in ("q", "k", "v", "p"):
                for t in range(2):
                    wbf[w, t] = consts.tile([128, C], BF16, tag=f"wbf{w}{t}", name=f"wbf{w}{t}")

            xq_f = [kqv.tile([128, NQ], F32, tag=f"xqf{t}", name=f"xqf{t}") for t in range(2)]
            k_sb = [kqv.tile([128, N], BF16, tag=f"k{t}", name=f"k{t}") for t in range(2)]
            q_sb = [kqv.tile([128, NQ], BF16, tag=f"q{t}", name=f"q{t}") for t in range(2)]
            vt1 = [kqv.tile([128, 2, C + 16], FP8, tag=f"vt{i}", name=f"vt{i}") for i in range(NT // 2)]
            a_t = [small.tile([128, 1], F32, tag=f"a{t}", name=f"a{t}") for t in range(2)]
            b_t = [small.tile([128, 1], F32, tag=f"b{t}", name=f"b{t}") for t in range(2)]
            cq = [small.tile([128, 1], F32, tag=f"cq{m}", name=f"cq{m}") for m in range(2)]
            cv = [small.tile([128, 1], F32, tag=f"cv{m}", name=f"cv{m}") for m in range(2)]
            sdum = small.tile([128, 1], F32, tag="sdum", name="sdum")
            ebias = small.tile([128, 1], F32, tag="ebias", name="ebias")
            nc.gpsimd.memset(ebias, -3.0)
            pdum = small.tile([128, 1], BF16, tag="pdum", name="pdum")

            with (
                tc.tile_pool(name="wraw", bufs=1) as wraw,
                tc.tile_pool(name="pspre", bufs=2, space="PSUM") as pspre,
            ):
                # PE warmup on the identity tile while DMAs stream
                wp_ps = pspre.tile([128, 512], F32, tag="warmps", name="warmps")
                for _ in range(WARMUP_MMS):
                    nc.tensor.matmul(wp_ps, lhsT=ident, rhs=warm_rhs, start=True, stop=True)
                # preload the sqrt ACT table before stats need it
                nc.scalar.sqrt(out=sdum, in_=ident[:, 0:1])

                wraw_t = {}
                for wname, dram in (("q", wqT), ("k", wkT), ("v", wvT), ("p", wpT)):
                    for t in range(2):
                        wt = wraw.tile([128, C], F32, tag=f"w{wname}{t}", name=f"w{wname}{t}")
                        nc.gpsimd.dma_start(out=wt, in_=dram[t * 128:(t + 1) * 128, :])
                        wraw_t[wname, t] = wt
                for t in range(2):
                    nc.gpsimd.dma_start(out=xq_f[t], in_=xq[t * 128:(t + 1) * 128, :])

                # ---- GN stats: per-channel mean/var -> per-group -> affine a,b ----
                st = [small.tile([128, 2], F32, tag=f"st{t}", name=f"st{t}") for t in range(2)]
                for t in range(2):
                    stats6 = small.tile([128, 8, 6], F32, tag="stats6", name="stats6", bufs=2)
                    mv = small.tile([128, 2], F32, tag="mv", name="mv", bufs=2)
                    xv = x_bf[t].rearrange("p (a b) -> p a b", b=512)
                    for sg in range(8):
                        nc.vector.bn_stats(out=stats6[:, sg, :], in_=xv[:, sg, :])
                    nc.vector.bn_aggr(out=mv, in_=stats6)
                    nc.vector.tensor_copy(out=st[t][:, 0:1], in_=mv[:, 0:1])
                    nc.vector.tensor_mul(out=st[t][:, 1:2], in0=mv[:, 0:1], in1=mv[:, 0:1])
                    nc.vector.tensor_add(out=st[t][:, 1:2], in0=st[t][:, 1:2], in1=mv[:, 1:2])

                ps_g = pspre.tile([G, 2], F32, tag="gstats", name="gstats")
                for t in range(2):
                    nc.tensor.matmul(ps_g, lhsT=ig_t[t], rhs=st[t],
                                     start=(t == 0), stop=(t == 1))
                # keep PE busy (HAM warm) while the DVE/ACT stats chain runs
                for _ in range(16):
                    nc.tensor.matmul(wp_ps, lhsT=ident, rhs=warm_rhs, start=True, stop=True)
                tg1 = small.tile([G, 1], F32, tag="tg1", name="tg1")
                tg2 = small.tile([G, 1], F32, tag="tg2", name="tg2")
                grs = small.tile([G, 2], F32, tag="grs", name="grs")
                nc.vector.tensor_copy(out=grs[:, 0:1], in_=ps_g[:, 0:1])
                nc.vector.tensor_mul(out=tg1, in0=grs[:, 0:1], in1=grs[:, 0:1])
                nc.vector.tensor_tensor(out=tg1, in0=ps_g[:, 1:2], in1=tg1,
                                        op=mybir.AluOpType.subtract)
                nc.vector.tensor_scalar_add(out=tg1, in0=tg1, scalar1=EPS)
                nc.vector.reciprocal(out=tg2, in_=tg1)
                nc.scalar.sqrt(out=grs[:, 1:2], in_=tg2)      # rsqrt(var+eps)

                for t in range(2):
                    mc = pspre.tile([128, 2], F32, tag="mcrs", name="mcrs")
                    nc.tensor.matmul(mc, lhsT=igt_sb[:, t * 128:(t + 1) * 128],
                                     rhs=grs, start=True, stop=True)
                    nc.vector.tensor_mul(out=a_t[t], in0=vecs_t[t][:, VG:VG + 1],
                                         in1=mc[:, 1:2])
                    nc.vector.tensor_mul(out=b_t[t], in0=mc[:, 0:1], in1=a_t[t])
                    nc.vector.tensor_tensor(out=b_t[t], in0=vecs_t[t][:, VB:VB + 1],
                                            in1=b_t[t], op=mybir.AluOpType.subtract)
                for _ in range(12):
                    nc.tensor.matmul(wp_ps, lhsT=ident, rhs=warm_rhs, start=True, stop=True)

                # fold GN scale into weight columns
                for w in ("q", "k", "v"):
                    for t in range(2):
                        nc.vector.tensor_scalar_mul(out=wbf[w, t], in0=wraw_t[w, t],
                                                    scalar1=a_t[t])
                for t in range(2):
                    nc.vector.tensor_copy(out=wbf["p", t], in_=wraw_t["p", t])

            # ---- K, V^T generation (evacs on ScalarE) ----
            with tc.tile_pool(name="psgen", bufs=1, space="PSUM") as psgen:
                kp = [psgen.tile([128, 512], F32, tag=f"g512_{n}", name=f"g512_{n}",
                                 bufs=1) for n in range(2)]
                for m in range(2):
                    for ng in range(4):
                        for t in range(2):
                            for n in range(2):
                                nn = ng * 2 + n
                                nc.tensor.matmul(kp[n],
                                                 lhsT=wbf["k", t][:, m * 128:(m + 1) * 128],
                                                 rhs=x_bf[t][:, nn * 512:(nn + 1) * 512],
                                                 start=(t == 0), stop=(t == 1))
                        for n in range(2):
                            nn = ng * 2 + n
                            nc.scalar.copy(out=k_sb[m][:, nn * 512:(nn + 1) * 512],
                                           in_=kp[n])
                for i in range(NT):
                    vp = psgen.tile([128, C], F32, tag="g256", name="g256", bufs=5)
                    for t in range(2):
                        nc.tensor.matmul(vp, lhsT=x_bf[t][:, i * 128:(i + 1) * 128],
                                         rhs=wbf["v", t], start=(t == 0), stop=(t == 1))
                    if i % 2 == 0:
                        nc.vector.tensor_copy(out=vt1[i // 2][:, 0, 0:C], in_=vp)
                    else:
                        nc.scalar.copy(out=vt1[i // 2][:, 1, 0:C], in_=vp)
                        nc.gpsimd.memset(vt1[i // 2][:, :, C:C + 16], 0.0)
                        nc.gpsimd.memset(vt1[i // 2][:, :, C:C + 1], 1.0)

                # bias constants (needed only at q-evac / ot-evac, so off the
                # critical path) and q generation
                for w, dst, bidx in (("q", cq, VBQ), ("v", cv, VBV)):
                    for m in range(2):
                        cp = psgen.tile([128, 1], F32, tag="cps", name="cps", bufs=1)
                        for t in range(2):
                            nc.tensor.matmul(cp, lhsT=wraw_t[w, t][:, m * 128:(m + 1) * 128],
                                             rhs=b_t[t], start=(t == 0), stop=(t == 1))
                        nc.vector.tensor_tensor(out=dst[m], in0=cp,
                                                in1=vecs_t[m][:, bidx:bidx + 1],
                                                op=mybir.AluOpType.add)
                for m in range(2):
                    for n in range(2):
                        qp = psgen.tile([128, 512], F32, tag="g512_0", name="qp", bufs=1)
                        for t in range(2):
                            nc.tensor.matmul(qp, lhsT=wbf["q", t][:, m * 128:(m + 1) * 128],
                                             rhs=xq_bf[t][:, n * 512:(n + 1) * 512],
                                             start=(t == 0), stop=(t == 1))
                        nc.vector.tensor_scalar_add(out=q_sb[m][:, n * 512:(n + 1) * 512],
                                                    in0=qp, scalar1=cq[m])
                # preload the exp ACT table while PE finishes generation
                nc.scalar.activation(out=pdum, in_=sdum,
                                     func=mybir.ActivationFunctionType.Exp, scale=1.0)

            # ---- S^T = K.T q ; P^T = exp(S^T/16), two nk-tiles per psum tile ----
            with tc.tile_pool(name="ptp", bufs=1) as ptp:
                pt = [ptp.tile([128, 2 * NQ], FP8, tag=f"pt{j}", name=f"pt{j}")
                      for j in range(NT // 2)]
                with tc.tile_pool(name="pss", bufs=2, space="PSUM") as pss:
                    for j in range(NT // 2):
                        sp = pss.tile([128, 2 * NQ], F32, tag="s", name="s")
                        for half in range(2):
                            i = 2 * j + half
                            for t in range(2):
                                for h in range(2):
                                    dst = sp[:, half * NQ + h * 512: half * NQ + (h + 1) * 512]
                                    nc.tensor.matmul(dst,
                                                     lhsT=k_sb[t][:, i * 128:(i + 1) * 128],
                                                     rhs=q_sb[t][:, h * 512:(h + 1) * 512],
                                                     start=(t == 0), stop=(t == 1))
                        nc.scalar.activation(out=pt[j], in_=sp, bias=ebias,
                                             func=mybir.ActivationFunctionType.Exp, scale=SCALE)
                    spw = pss.tile([128, 2 * NQ], F32, tag="s", name="spw")
                    for _ in range(8):
                        nc.tensor.matmul(spw[:, 0:512], lhsT=ident, rhs=warm_rhs,
                                         start=True, stop=True)

                # ---- O = P^T.T @ [V^T | 1]; normalize; transpose; project ----
                with (
                    tc.tile_pool(name="pso", bufs=4, space="PSUM") as pso,
                    tc.tile_pool(name="pst", bufs=2, space="PSUM") as pst,
                    tc.tile_pool(name="psy", bufs=2, space="PSUM") as psy,
                ):
                    o_sb = [attn.tile([128, C], BF16, tag=f"o{j}", name=f"o{j}") for j in range(NQT)]
                    ot = [attn.tile([128, NQ], BF16, tag=f"ot{t}", name=f"ot{t}") for t in range(2)]
                    y_sb = [attn.tile([128, NQ], F32, tag=f"y{t}", name=f"y{t}") for t in range(2)]

                    for j in range(NQT):
                        op_ = pso.tile([128, C + 1], F32, tag="o", name="o")
                        for jp in range(NT // 2):
                            lhsT = pt[jp].rearrange("p (ko q) -> p ko q", ko=2)[:, :, j * 128:(j + 1) * 128]
                            nc.tensor.matmul(op_, lhsT=lhsT, rhs=vt1[jp][:, :, 0:C + 1],
                                             start=(jp == 0), stop=(jp == NT // 2 - 1),
                                             perf_mode=mybir.MatmulPerfMode.DoubleRow)
                        rec = small.tile([128, 1], F32, tag="rec", name="rec", bufs=3)
                        nc.vector.reciprocal(out=rec, in_=op_[:, C:C + 1])
                        nc.vector.tensor_scalar_mul(out=o_sb[j], in0=op_[:, 0:C], scalar1=rec)
                        for t in range(2):
                            tp = pst.tile([128, 128], BF16, tag="tp", name="tp")
                            nc.tensor.transpose(tp, o_sb[j][:, t * 128:(t + 1) * 128], ident)
                            nc.vector.tensor_scalar_add(out=ot[t][:, j * 128:(j + 1) * 128],
                                                        in0=tp, scalar1=cv[t])
                        if j % 4 == 3:
                            n = j // 4
                            for m in range(2):
                                yp = psy.tile([128, 512], F32, tag="y", name="yps")
                                for t in range(2):
                                    nc.tensor.matmul(yp, lhsT=wbf["p", t][:, m * 128:(m + 1) * 128],
                                                     rhs=ot[t][:, n * 512:(n + 1) * 512],
                                                     start=(t == 0), stop=(t == 1))
                                    
                                nc.scalar.add(out=y_sb[m][:, n * 512:(n + 1) * 512],
                                                          in_=yp, add=vecs_t[m][:, VBP:VBP + 1])
                                nc.vector.tensor_add(out=y_sb[m][:, n * 512:(n + 1) * 512],
                                                     in0=y_sb[m][:, n * 512:(n + 1) * 512],
                                                     in1=xq_f[m][:, n * 512:(n + 1) * 512])
                                nc.sync.dma_start(out=y[m, n],
                                                  in_=y_sb[m][:, n * 512:(n + 1) * 512])

    nc.compile()
    return nc


_NC_CACHE = None


def _get_nc():
    global _NC_CACHE
    if _NC_CACHE is None:
        _NC_CACHE = build_nc()
    return _NC_CACHE


def make_in_maps(inputs):
    x = np.ascontiguousarray(np.asarray(inputs["x"], np.float32))
    xf = x.reshape(B, C, N)
    xf_bf = xf.astype(ml_dtypes.float8_e4m3)
    group = np.arange(C) // (C // G)  # channel -> group
    ig = np.zeros((2, 128, G), np.float32)
    igt = np.zeros((G, C), np.float32)
    for c in range(C):
        ig[c // 128, c % 128, group[c]] = 1.0 / (C // G)
        igt[group[c], c] = 1.0
    vecs = np.zeros((2, 128, 5), np.float32)
    for t in range(2):
        sl = slice(t * 128, (t + 1) * 128)
        vecs[t, :, VG] = np.asarray(inputs["gn_gamma"])[sl]
        vecs[t, :, VB] = np.asarray(inputs["gn_beta"])[sl]
        vecs[t, :, VBQ] = np.asarray(inputs["bq"])[sl]
        vecs[t, :, VBV] = np.asarray(inputs["bv"])[sl]
        vecs[t, :, VBP] = np.asarray(inputs["bp"])[sl]
    common = {
        "wqT": np.ascontiguousarray(np.asarray(inputs["Wq"], np.float32).T),
        "wkT": np.ascontiguousarray(np.asarray(inputs["Wk"], np.float32).T),
        "wvT": np.ascontiguousarray(np.asarray(inputs["Wv"], np.float32).T),
        "wpT": np.ascontiguousarray(np.asarray(inputs["Wp"], np.float32).T),
        "vecs": vecs, "ig": ig, "igt": igt,
    }
    in_maps = []
    for core in range(8):
        b, ch = core // 4, core % 4
        xb_cm = np.ascontiguousarray(
            xf_bf[b].reshape(2, 128, 4, 1024).transpose(0, 2, 1, 3))
        in_maps.append({
            "xb": xb_cm,
            "xqb": np.ascontiguousarray(xf_bf[b][:, ch * NQ:(ch + 1) * NQ]),
            "xq": np.ascontiguousarray(xf[b][:, ch * NQ:(ch + 1) * NQ]),
            **common,
        })
    return in_maps, x


def run(inputs, trace=False, tmpdir=None):
    nc = _get_nc()
    in_maps, x = make_in_maps(inputs)
    res = run_bass_kernel_spmd(nc, in_maps, core_ids=list(range(8)),
                               trace=trace, tmpdir=tmpdir)
    out = np.empty((B, C, N), np.float32)
    for core in range(8):
        b, ch = core // 4, core % 4
        yc = res.results[core]["y"]  # [2, 2, 128, 512] -> [256, 1024]
        out[b][:, ch * NQ:(ch + 1) * NQ] = yc.transpose(0, 2, 1, 3).reshape(C, NQ)
    return out.reshape(B, C, 16, 16, 16), res


def kernel(**inputs) -> np.ndarray:
    out, _ = run(inputs, trace=False)
    return out

